# revision 8
# baseline (speedup 1.0000x reference)
"""Multi-head channel-attention kernel for Trainium2 (8 NeuronCores, SPMD).

Reference computation (per batch b, x = [256, N] with N = 64*64 = 4096):
    qkv   = w_qkv @ x
    q,k,v = per-head [256, N] slices of qkv
    logit = (q*scale) @ k.T          # [256, 256] (contraction over N)
    wts   = softmax(logit, -1)
    out_h = wts @ v
    y     = w_out @ stack_h(out_h) + b_out

Distribution: pure data-parallel — batch 8 across 8 cores, one batch per
core, no collectives.

The kernel exploits that attention is over the *channel* axis (n >> c):

    logit_h = (Wq_h * scale) @ (x @ x.T) @ Wk_h.T
    y       = (sum_h W_h @ softmax_h @ Wv_h) @ x + b  =  Wstar @ x + b

so the only n-wide work is the Gram matrix G = x @ x.T (one pass over x)
and the final Wstar @ x (second pass). Everything else is [256,256]-sized.
Per-batch FLOPs drop from 12.9G (direct) to 1.6G.

All operands are fp16 (not bf16): same PE speed and DMA bytes, 4x the
mantissa. y is stored fp16 and widened on the host, halving the output
flush (the baseline's 9us tail was the single biggest cost).

Pipeline (all matmuls TensorE, fp16 operands, fp32 PSUM):
    G     = xT.T @ xT                 (xT shipped pre-transposed from host)
    heads are processed in 2 groups of 2 so every non-PE op is 512 wide:
    A_g   = G @ WkT_g                 -> [256, 512] PSUM, drained fp16
    L_g   = (Wq_g*scale) @ A_g        -> PSUM
    E_g   = exp(L_g) via one wide ACT activation per [128,512] tile
            (table preloaded by a dummy exp at kernel start); row sums via
            DVE tensor_reduce, reciprocal, in-place E scale (DVE/Pool)
    M_g   = Ehat_g . WoT_g            (computed directly transposed)
    WstarT= sum_g Wv_g-contraction with M_g
    y     = WstarT.T @ x + b          (bias folded into the PSUM drains,
            which alternate DVE/ACT so they keep up with the PE)

Engine budget for the middle phase (the baseline's bottleneck): drains are
split ACT/DVE per tile, exps on ACT, softmax arithmetic on DVE with the
E-scales split DVE/Pool, so no single engine exceeds the PE's ~6.7us.

Weights are repacked/pre-transposed on the host; x is shipped twice (native
and transposed, fp16) so the Gram matmul needs no on-chip transpose.
"""

import numpy as np

import concourse.bass as bass
import concourse.mybir as mybir
import concourse.tile as tile
from concourse.bass import ts
from concourse.bass_utils import run_bass_kernel_spmd
from concourse.vector_clock import ScopedClock

B, DIM, H, W = 8, 256, 64, 64
HEADS = 4
N = H * W            # 4096
P = 128
KT = DIM // P        # 2 channel tiles
NT = N // P          # 32 n-tiles of 128
NQ = 8               # xT shipped in 8 slabs of 4 n-tiles
NCH = N // 512       # 8 n-chunks of 512
N_CORES = 8
NG = 2               # head groups (2 heads each -> 512-wide tiles)
HPG = HEADS // NG

F32 = mybir.dt.float32
FP16 = mybir.dt.float16
NPFP16 = np.float16


def _split_multi_waits(nc, max_waits=1):
    """The walrus build in this container rejects instructions carrying more
    than one sync-wait. Move excess waits onto same-engine carrier NOPs
    inserted immediately before the instruction (engines are in-order, so
    waiting earlier on the same stream is equivalent)."""
    n_split = 0
    for f in nc.m.functions:
        for bb in f.blocks:
            old = list(bb.instructions)
            new = []
            changed = False
            for inst in old:
                si = inst.sync_info
                waits = list(si.on_wait) if si and si.on_wait else []
                if len(waits) > max_waits:
                    changed = True
                    for w in waits[max_waits:]:
                        n_split += 1
                        new.append(
                            mybir.InstNoOp(
                                name=f"wsplit_{n_split}_{inst.name}",
                                engine=inst.engine,
                                ins=[],
                                outs=[],
                                sync_info=mybir.SyncInfo(on_wait=[w], on_update=[]),
                            )
                        )
                    inst.sync_info = mybir.SyncInfo(
                        on_wait=waits[:max_waits], on_update=si.on_update
                    )
                new.append(inst)
            if changed:
                bb.instructions = new
    return n_split


def _minimal_exit(self, tick_clock, wait_clock):
    """TileContext._drain_and_barrier replacement: one SP drain carrying the
    global-clock waits (split onto NOPs by _split_multi_waits afterwards).

    The stock exit adds two all-engine barriers and ~200 per-semaphore
    clears (~10 us). They are redundant here: the bass preamble range-clears
    the whole kernel semaphore range at startup, and bass's own postamble
    still drains every engine.
    """
    nc = self.nc
    drain = nc.sync.drain()
    wait_clock.add_sem_waits(drain.ins, ScopedClock({None: tick_clock.global_clock}))
    popped = nc._tile_sem_poison_stack.pop()
    assert popped is self._sem_poison


def build_program():
    """Build the single-core Bass program (run SPMD across 8 cores)."""
    nc = bass.Bass()

    x_d = nc.declare_dram_parameter("x", [DIM, N], FP16, isOutput=False)
    # xt: [NQ][128, 4, 256]; slab qi, element (p, a, c) = x.T[qi*512 + a*128 + p, c]
    xt_d = nc.declare_dram_parameter("xt", [NQ, P, NT // NQ, DIM], FP16, isOutput=False)
    # wkq: [KT][128, 2048] = [wqT | wkT]; wvo: [KT][128, 2048] = [wv | woT]
    wkq_d = nc.declare_dram_parameter("wkq", [KT, P, 2 * HEADS * DIM], FP16, isOutput=False)
    wvo_d = nc.declare_dram_parameter("wvo", [KT, P, 2 * HEADS * DIM], FP16, isOutput=False)
    b_d = nc.declare_dram_parameter("b", [DIM, 1], F32, isOutput=False)
    y_d = nc.declare_dram_parameter("y", [DIM, N], FP16, isOutput=True)

    prev_exit = tile.TileContext._drain_and_barrier
    tile.TileContext._drain_and_barrier = _minimal_exit
    try:
        _build_body(nc, x_d, xt_d, wkq_d, wvo_d, b_d, y_d)
    finally:
        tile.TileContext._drain_and_barrier = prev_exit

    _split_multi_waits(nc)
    return nc


def _build_body(nc, x_d, xt_d, wkq_d, wvo_d, b_d, y_d):
    OQT, OKT, OV, OOT = 0, HEADS * DIM, 0, HEADS * DIM
    EXP = mybir.ActivationFunctionType.Exp
    with tile.TileContext(nc) as tc:
        with (
            tc.tile_pool(name="wpool", bufs=1) as wpool,
            tc.tile_pool(name="spool", bufs=1) as spool,
            tc.tile_pool(name="ypool", bufs=1) as ypool,
            tc.tile_pool(name="psum", bufs=1, space="PSUM") as psum,
        ):
            # ---- PE warmup: dummy matmuls during the input DMAs release
            # the HAM clock-gate so G runs at full clock from its first
            # instruction. Shares the pG psum tag (bank budget is exactly 8).
            warm = wpool.tile([P, P], FP16, tag="warm")
            nc.gpsimd.memset(warm[:], 0)
            wps = psum.tile([P, 2 * DIM], F32, tag="pW", bufs=1, name="warmps")
            for _ in range(24):
                nc.tensor.matmul(wps[:, 0:P], warm[:], warm[:], start=True, stop=True)

            # ---- loads (xT slabs first: G consumes them incrementally;
            # triggers split across the two HWDGE engines (SP + ACT) so the
            # trigger chains run in parallel) ----
            xt_sb = []
            for qi in range(NQ):
                t = wpool.tile([P, NT // NQ, DIM], FP16, tag=f"xt{qi}")
                eng = nc.sync if qi % 2 == 0 else nc.scalar
                eng.dma_start(t[:], xt_d[qi])
                xt_sb.append(t)
            wkq_sb = []
            for k in range(KT):
                t = wpool.tile([P, 2 * HEADS * DIM], FP16, tag=f"wkq{k}")
                eng = nc.sync if k == 0 else nc.scalar
                eng.dma_start(t[:], wkq_d[k])
                wkq_sb.append(t)
            wvo_sb = []
            for k in range(KT):
                t = wpool.tile([P, 2 * HEADS * DIM], FP16, tag=f"wvo{k}")
                eng = nc.sync if k == 0 else nc.scalar
                eng.dma_start(t[:], wvo_d[k])
                wvo_sb.append(t)
            x_sb = []
            for k in range(KT):
                t = wpool.tile([P, N], FP16, tag=f"x{k}")
                eng = nc.sync if k == 0 else nc.scalar
                eng.dma_start(t[:], x_d[ts(k, P), :])
                x_sb.append(t)
            b_sb = []
            for ot in range(KT):
                t = wpool.tile([P, 1], F32, tag=f"b{ot}")
                eng = nc.sync if ot == 0 else nc.scalar
                eng.dma_start(t[:], b_d[ts(ot, P), :])
                b_sb.append(t)

            # ---- ACT exp-table preload: a dummy exp right after the scalar
            # engine's DMA triggers hides the ~1.3us ACT_TABLE_LOAD that
            # otherwise lands in front of the first real softmax.
            warm_e = spool.tile([P, 1], FP16, tag="warme")
            nc.scalar.activation(warm_e[:], warm[:, 0:1], EXP)

            # ---- G = x @ x.T (fp32 PSUM, 32 accumulation steps) ----------
            # both ct-tiles packed into one PSUM bank ([128,512] f32)
            g_bank = psum.tile([P, 2 * DIM], F32, tag="pG", bufs=1, name="gbank")
            g_ps = [g_bank[:, ts(ct, DIM)] for ct in range(KT)]
            for i in range(NT):
                qi, a = divmod(i, NT // NQ)
                for ct in range(KT):
                    nc.tensor.matmul(
                        g_ps[ct],
                        xt_sb[qi][:, a, ts(ct, P)],
                        xt_sb[qi][:, a, :],
                        start=(i == 0 and ct == 0),
                        stop=(i == NT - 1 and ct == KT - 1),
                    )
            g_sb = []
            for ct in range(KT):
                g = spool.tile([P, DIM], FP16, tag=f"gs{ct}", name=f"g{ct}")
                eng = nc.scalar.copy if ct == 0 else nc.vector.tensor_copy
                eng(g[:], g_ps[ct])
                g_sb.append(g)

            # ---- head-group pipeline: groups of 2 heads -> 512-wide tiles -
            # stage A(g): A = G @ WkT_g           [2ct][128, 512] PSUM
            # stage L(g): L = (Wq_g*scale) @ A    [2ct][128, 512] PSUM
            #             E = exp(L) wide ACT; rowsum/recip/scale DVE+Pool
            # stage M(g): M_gT = Ehat_g . WoT_g   [2dt][128, 512] PSUM
            GW = HPG * DIM  # 512: group width

            at = {}    # (g, k)   -> A SBUF fp16 [128, 512]
            e_sb = {}  # (g, ct)  -> Ehat SBUF fp16 [128, 512]
            mt = {}    # (g, dt2) -> M SBUF fp16 [128, 512]

            def stage_A(g):
                for ct in range(KT):
                    ap = psum.tile([P, GW], F32, tag="pA", bufs=2, name=f"pa{g}_{ct}")
                    for k in range(KT):
                        nc.tensor.matmul(
                            ap[:],
                            g_sb[k][:, ts(ct, P)],
                            wkq_sb[k][:, OKT + g * GW : OKT + (g + 1) * GW],
                            start=(k == 0),
                            stop=(k == KT - 1),
                        )
                    t = spool.tile([P, GW], FP16, tag=f"at{ct}", bufs=2, name=f"at{g}_{ct}")
                    eng = nc.scalar.copy if ct == 0 else nc.vector.tensor_copy
                    eng(t[:], ap[:])
                    at[(g, ct)] = t

            def stage_L(g):
                # one [128,512] PSUM tile per ct; exp immediately after each
                for ct in range(KT):
                    lp = psum.tile([P, GW], F32, tag="pL", bufs=2, name=f"pl{g}_{ct}")
                    for hh in range(HPG):
                        h = g * HPG + hh
                        for k in range(KT):
                            nc.tensor.matmul(
                                lp[:, hh * DIM : (hh + 1) * DIM],
                                wkq_sb[k][:, OQT + h * DIM + ct * P : OQT + h * DIM + (ct + 1) * P],
                                at[(g, k)][:, hh * DIM : (hh + 1) * DIM],
                                start=(hh == 0 and k == 0),
                                stop=(hh == HPG - 1 and k == KT - 1),
                            )
                    e = spool.tile([P, GW], FP16, tag=f"e{ct}", bufs=2, name=f"e{g}_{ct}")
                    nc.scalar.activation(e[:], lp[:], EXP)
                    e_sb[(g, ct)] = e
                # softmax denominators: DVE rowsum + reciprocal, then scale
                # E in place (split DVE / Pool so the last scale lands fast)
                for ct in range(KT):
                    e = e_sb[(g, ct)]
                    s = spool.tile([P, HPG], F32, tag=f"s{ct}", bufs=2, name=f"s{g}_{ct}")
                    r = spool.tile([P, HPG], F32, tag=f"r{ct}", bufs=2, name=f"r{g}_{ct}")
                    for hh in range(HPG):
                        nc.vector.tensor_reduce(
                            s[:, hh : hh + 1],
                            e[:, hh * DIM : (hh + 1) * DIM],
                            mybir.AxisListType.X,
                            mybir.AluOpType.add,
                        )
                    nc.vector.reciprocal(r[:], s[:])
                    for hh in range(HPG):
                        eng = nc.vector if hh % 2 == 0 else nc.gpsimd
                        eng.tensor_scalar_mul(
                            e[:, hh * DIM : (hh + 1) * DIM],
                            e[:, hh * DIM : (hh + 1) * DIM],
                            r[:, hh : hh + 1],
                        )

            def stage_M(g):
                for dt2 in range(KT):
                    pm = psum.tile([P, GW], F32, tag="pM", bufs=2, name=f"pm{g}_{dt2}")
                    for hh in range(HPG):
                        h = g * HPG + hh
                        for ct in range(KT):
                            nc.tensor.matmul(
                                pm[:, hh * DIM : (hh + 1) * DIM],
                                e_sb[(g, ct)][:, hh * DIM + dt2 * P : hh * DIM + (dt2 + 1) * P],
                                wvo_sb[ct][:, OOT + h * DIM : OOT + (h + 1) * DIM],
                                start=(hh == 0 and ct == 0),
                                stop=(hh == HPG - 1 and ct == KT - 1),
                            )
                    t = spool.tile([P, GW], FP16, tag=f"mt{dt2}", bufs=2, name=f"mt{g}_{dt2}")
                    eng = nc.scalar.copy if dt2 == 0 else nc.vector.tensor_copy
                    eng(t[:], pm[:])
                    mt[(g, dt2)] = t

            # ---- WstarT[c_in, o] accumulates over (g, h, dt2) ------------
            # both ct-tiles packed into the warmup's PSUM bank
            wst_ps = [wps[:, ts(ct, DIM)] for ct in range(KT)]

            def stage_W(g):
                for ct in range(KT):
                    for hh in range(HPG):
                        h = g * HPG + hh
                        for dt2 in range(KT):
                            nc.tensor.matmul(
                                wst_ps[ct],
                                wvo_sb[dt2][:, OV + h * DIM + ct * P : OV + h * DIM + (ct + 1) * P],
                                mt[(g, dt2)][:, hh * DIM : (hh + 1) * DIM],
                                start=(ct == 0 and g == 0 and hh == 0 and dt2 == 0),
                                stop=(ct == KT - 1 and g == NG - 1 and hh == HPG - 1 and dt2 == KT - 1),
                            )

            # pipelined emission: PE order A0 A1 L0 L1 M0 W0 M1 W1
            stage_A(0)
            stage_A(1)
            stage_L(0)
            stage_L(1)
            stage_M(0)
            stage_W(0)
            stage_M(1)
            stage_W(1)

            wst_sb = []
            for ct in range(KT):
                wt = spool.tile([P, DIM], FP16, tag=f"wst{ct}", name=f"wt{ct}")
                eng = nc.scalar.copy if ct == 0 else nc.vector.tensor_copy
                eng(wt[:], wst_ps[ct])
                wst_sb.append(wt)

            # ---- y = WstarT.T @ x + b ------------------------------------
            # drains alternate DVE/ACT; fp16 output, 6 output DMAs
            y_sb = {}
            for ot in range(KT):
                y_sb[ot] = ypool.tile([P, N], FP16, tag=f"y{ot}", name=f"ysb{ot}")
            store_after = {2: (0, 3), 5: (3, 3), 7: (6, 2)}
            ycnt = 0
            ytags = ["pA", "pA", "pL", "pL", "pM", "pM"]
            for j in range(NCH):
                for ot in range(KT):
                    py = psum.tile([P, 512], F32, tag=ytags[ycnt % 6], bufs=2,
                                   name=f"py{j}_{ot}")
                    for k in range(KT):
                        nc.tensor.matmul(
                            py[:],
                            wst_sb[k][:, ts(ot, P)],
                            x_sb[k][:, ts(j, 512)],
                            start=(k == 0),
                            stop=(k == KT - 1),
                        )
                    dst = y_sb[ot][:, ts(j, 512)]
                    if ycnt % 2 == 0:
                        nc.vector.tensor_scalar_add(dst, py[:], b_sb[ot][:])
                    else:
                        nc.scalar.add(dst, py[:], b_sb[ot][:])
                    ycnt += 1
                    if j in store_after:
                        j0, nj = store_after[j]
                        nc.sync.dma_start(
                            y_d[ts(ot, P), j0 * 512 : (j0 + nj) * 512],
                            y_sb[ot][:, j0 * 512 : (j0 + nj) * 512],
                        )


def prep_inputs(x, w_qkv, w_out, b_out):
    """Host-side packing: per-core input dicts (numpy only)."""
    x = np.asarray(x, dtype=np.float32)
    w_qkv = np.asarray(w_qkv, dtype=np.float32)
    w_out = np.asarray(w_out, dtype=np.float32)
    b_out = np.asarray(b_out, dtype=np.float32)

    scale = float(DIM) ** -0.5
    wq = w_qkv[0 * HEADS * DIM : 1 * HEADS * DIM].reshape(HEADS, DIM, DIM)
    wk = w_qkv[1 * HEADS * DIM : 2 * HEADS * DIM].reshape(HEADS, DIM, DIM)
    wv = w_qkv[2 * HEADS * DIM : 3 * HEADS * DIM].reshape(HEADS, DIM, DIM)

    # wqT[c', h*256 + c] = wq[h, c, c'] * scale
    wqT = (np.transpose(wq, (2, 0, 1)) * scale).reshape(DIM, HEADS * DIM)
    # wkT[c', h*256 + d] = wk[h, d, c']
    wkT = np.transpose(wk, (2, 0, 1)).reshape(DIM, HEADS * DIM)
    # wvn[d, h*256 + c_in] = wv[h, d, c_in]  (natural orientation, head-concat)
    wvn = np.transpose(wv, (1, 0, 2)).reshape(DIM, HEADS * DIM)
    # woT[c, h*256 + o] = w_out[o, c*HEADS + h]
    woT = np.ascontiguousarray(
        w_out.reshape(DIM, DIM, HEADS).transpose(1, 2, 0)
    ).reshape(DIM, HEADS * DIM)

    # wkq[k] = [wqT | wkT], wvo[k] = [wv | woT], rows k*128:(k+1)*128
    wkq = np.ascontiguousarray(
        np.concatenate([wqT, wkT], axis=1).astype(NPFP16).reshape(KT, P, 2 * HEADS * DIM)
    )
    wvo = np.ascontiguousarray(
        np.concatenate([wvn, woT], axis=1).astype(NPFP16).reshape(KT, P, 2 * HEADS * DIM)
    )
    b = b_out.reshape(DIM, 1).astype(np.float32)

    in_maps = []
    for bi in range(B):
        xb = np.ascontiguousarray(x[bi].reshape(DIM, N)).astype(NPFP16)
        # xt[qi, p, a, c] = x.T[qi*512 + a*128 + p, c]
        xt = np.ascontiguousarray(
            xb.T.reshape(NQ, NT // NQ, P, DIM).transpose(0, 2, 1, 3)
        )
        in_maps.append({"x": xb, "xt": xt, "wkq": wkq, "wvo": wvo, "b": b})
    return in_maps


_NC_CACHE = {}


def get_program():
    if "nc" not in _NC_CACHE:
        _NC_CACHE["nc"] = build_program()
    return _NC_CACHE["nc"]


def kernel(x, w_qkv, w_out, b_out, **_unused):
    nc = get_program()
    in_maps = prep_inputs(x, w_qkv, w_out, b_out)
    res = run_bass_kernel_spmd(nc, in_maps, list(range(N_CORES)))
    y = np.stack([res.results[c]["y"] for c in range(N_CORES)], axis=0)
    return y.reshape(B, DIM, H, W).astype(np.float32)
